# revision 2
# baseline (speedup 1.0000x reference)
"""Transformer-XL relative multi-head attention, 8-way sharded on Trainium2.

Self-contained harness entry: kernel(**inputs) -> np.ndarray [4, 1024, 1024].

Sharding: core c handles batch b = c//2 and head-half hh = c%2 (8 of 16
heads). Each core computes a partial output (its heads' contribution
through Wo); the host unshard sums the two partials per batch (row-parallel
tensor parallelism for the output projection).
"""

import os
import sys

sys.path.insert(0, "/opt/trn_rl_repo")

import numpy as np


import concourse.bass as bass
import concourse.mybir as mybir
from concourse.tile import TileContext, ScopedClock

F32 = mybir.dt.float32
F32R = mybir.dt.float32r
F16 = mybir.dt.float16
AF = mybir.ActivationFunctionType
OP = mybir.AluOpType

S, T, D, HC, DK, P = 1024, 2048, 1024, 8, 64, 128
DH = HC * DK  # 512, head-slice width per core
NQT = S // P  # 8 query tiles
WMAX = 2048 + 127 + 128  # padded shear slot width (>= max W)
SLOT = P * WMAX + P  # dram scratch slot elements (pad for strided read)
EXP_BIAS = -7.0
NEG_BIG = -60000.0


def _patched_drain_and_barrier(self, tick_clock, wait_clock):
    # The walrus build in this container caps sync-waits per instruction;
    # Tile's stock tail drain carries one wait per live proc. Emit one SP nop
    # per wait instead, then the drain.
    dummy = mybir.InstNoOp(name="drain-wait-probe", ins=[], outs=[])
    dummy.engine = mybir.EngineType.SP
    wait_clock.add_sem_waits(dummy, ScopedClock({None: tick_clock.global_clock}))
    waits = []
    if dummy.sync_info is not None and dummy.sync_info.on_wait:
        waits = [(w.ant_name, w.wait_value) for w in dummy.sync_info.on_wait]
    assert self.sems is not None
    name2sem = {h.name: h for h in self.sems.allocated().values()}
    for name, val in waits:
        self.nc.sync.nop().wait_op(name2sem[name], val, "sem-ge")
    self.nc.sync.drain()
    self.nc.all_engine_barrier()
    popped = self.nc._tile_sem_poison_stack.pop()
    assert popped is self._sem_poison
    self.nc.clear_and_free_semaphores(list(self.sems.allocated().values()))
    self.nc.all_engine_barrier()


TileContext._drain_and_barrier = _patched_drain_and_barrier



def _split_multi_waits(nc, max_waits=1):
    """Walrus in this container rejects instructions carrying more than a
    couple of sync waits. Hoist extras onto same-engine NoOps just before
    the instruction (sequential on the engine, so semantics unchanged)."""
    for f in nc.m.functions:
        for bb in f.blocks:
            out = []
            changed = False
            for inst in bb.instructions:
                si = inst.sync_info
                if si is not None and si.on_wait and len(si.on_wait) > max_waits:
                    waits = list(si.on_wait)
                    for j, w in enumerate(waits[:-max_waits]):
                        nop = mybir.InstNoOp(
                            name=f"{inst.name}-wsplit{j}", ins=[], outs=[])
                        nop.engine = inst.engine
                        nop.sync_info = mybir.SyncInfo(on_wait=[w], on_update=[])
                        out.append(nop)
                    inst.sync_info = mybir.SyncInfo(
                        on_wait=waits[-max_waits:],
                        on_update=list(si.on_update))
                    changed = True
                out.append(inst)
            if changed:
                bb.instructions = out


def kq_of(qi):  # valid key count for query tile qi (keys j <= i + 1024)
    return (qi + 9) * P


def build_nc(split_waits=True):
    nc = bass.Bass(target_bir_lowering=True)

    # fp32 inputs are declared float32r: same bits, PE runs the fp22
    # full-rate path on them.
    qT = nc.declare_dram_parameter("qT", [D, S], F32R, isOutput=False)
    kT = nc.declare_dram_parameter("kT", [D, T], F32R, isOutput=False)
    vT = nc.declare_dram_parameter("vT", [D, T], F32R, isOutput=False)
    RT = nc.declare_dram_parameter("RT", [D, T], F32R, isOutput=False)
    Wq = nc.declare_dram_parameter("Wq", [D, DH], F32R, isOutput=False)
    Wk = nc.declare_dram_parameter("Wk", [D, DH], F32R, isOutput=False)
    Wv = nc.declare_dram_parameter("Wv", [D, DH], F32R, isOutput=False)
    Wr = nc.declare_dram_parameter("Wr", [D, DH], F32R, isOutput=False)
    Wo16 = nc.declare_dram_parameter("Wo16", [DH, D], F16, isOutput=False)
    ub = nc.declare_dram_parameter("ub", [P, 4], F32, isOutput=False)
    vb = nc.declare_dram_parameter("vb", [P, 4], F32, isOutput=False)
    atril = nc.declare_dram_parameter("atril", [P, P], mybir.dt.uint8, isOutput=False)
    outp = nc.declare_dram_parameter("out", [S, D], F32, isOutput=True)

    with TileContext(nc) as tc:
        with (
            tc.tile_pool(name="persist", bufs=1) as pp,
            tc.tile_pool(name="consts", bufs=1) as cp,
        ):
            # persistent fp16 tensors (partition = dk within head-pair tile)
            quT = pp.tile([P, 4 * S], F16)      # (qh+u).T   blocks hp
            qvT = pp.tile([P, 4 * S], F16)      # (qh+v).T
            khT = pp.tile([P, 4 * T], F16)
            rh2T = pp.tile([P, 4 * 3072], F16)
            vh16 = pp.tile([P, 16 * (HC * 65)], F16)  # per key tile: 8 heads x (64+1)
            concatT = pp.tile([P, 4 * S], F16)
            WoS = pp.tile([P, 4 * D], F16)

            ub_sb = cp.tile([P, 4], F32)
            vb_sb = cp.tile([P, 4], F32)
            atril_sb = cp.tile([P, P], mybir.dt.uint8)
            negbig_sb = cp.tile([P, P], F32)
            expb_sb = cp.tile([P, 1], F32)
            nc.vector.memset(expb_sb[:], EXP_BIAS)

            nc.sync.dma_start(out=ub_sb[:], in_=ub[:])
            nc.sync.dma_start(out=vb_sb[:], in_=vb[:])
            nc.sync.dma_start(out=atril_sb[:], in_=atril[:])
            nc.vector.memset(negbig_sb[:], NEG_BIG)
            # WoS layout [128, dt*1024 + o] <- Wo16[(dt p), o]
            for dt_ in range(4):
                nc.sync.dma_start(
                    out=WoS[:, dt_ * D : (dt_ + 1) * D],
                    in_=Wo16[dt_ * P : (dt_ + 1) * P, :],
                )

            # ---------------- projections ----------------
            def load_w(pool, wparam):
                wsb = pool.tile([P, 8 * DH], F32R, tag="wsb")
                for kd in range(8):
                    nc.sync.dma_start(
                        out=wsb[:, kd * DH : (kd + 1) * DH],
                        in_=wparam[kd * P : (kd + 1) * P, :],
                    )
                return wsb

            # qhT-style projection: out[512, ncols] = W_s @ xT, evacuated by fn
            def proj_T(pool, psum, wsb, xparam, ncols, evac):
                nth = ncols // 1024
                for th in range(nth):
                    psums = {k: psum.tile([P, 512], F32, tag="proj", name="projps")
                             for k in [(d, t2) for d in range(4) for t2 in range(2)]}
                    for kd in range(8):
                        xsb = pool.tile([P, 1024], F32R, tag="xstage")
                        nc.sync.dma_start(
                            out=xsb[:],
                            in_=xparam[kd * P : (kd + 1) * P,
                                       th * 1024 : (th + 1) * 1024],
                        )
                        for dot in range(4):
                            for tc2 in range(2):
                                nc.tensor.matmul(
                                    psums[(dot, tc2)][:],
                                    wsb[:, kd * DH + dot * P : kd * DH + (dot + 1) * P],
                                    xsb[:, tc2 * 512 : (tc2 + 1) * 512],
                                    start=(kd == 0),
                                    stop=(kd == 7),
                                )
                    for dot in range(4):
                        for tc2 in range(2):
                            evac(psums[(dot, tc2)], dot, th * 1024 + tc2 * 512)

            with (
                tc.tile_pool(name="projp", bufs=3) as jp,
                tc.tile_pool(name="projw", bufs=2) as jw,
                tc.tile_pool(name="rhtmp", bufs=1) as jr,
                tc.tile_pool(name="projpsum", bufs=8, space="PSUM") as jps,
            ):
                wsb = load_w(jw, Wq)

                def evac_q(ps, dot, col):
                    nc.vector.tensor_scalar(
                        quT[:, dot * S + col : dot * S + col + 512], ps[:],
                        ub_sb[:, dot : dot + 1], None, OP.add)
                    nc.vector.tensor_scalar(
                        qvT[:, dot * S + col : dot * S + col + 512], ps[:],
                        vb_sb[:, dot : dot + 1], None, OP.add)

                proj_T(jp, jps, wsb, qT, S, evac_q)

                wsb = load_w(jw, Wk)

                def evac_k(ps, dot, col):
                    nc.vector.tensor_copy(
                        khT[:, dot * T + col : dot * T + col + 512], ps[:])

                proj_T(jp, jps, wsb, kT, T, evac_k)

                rhT = jr.tile([P, 4 * T], F16, tag="rhT")
                wsb = load_w(jw, Wr)

                def evac_r(ps, dot, col):
                    nc.vector.tensor_copy(
                        rhT[:, dot * T + col : dot * T + col + 512], ps[:])

                proj_T(jp, jps, wsb, RT, T, evac_r)

                # rh2T[:, m'] = rhT[:, (m' + 1023) % 2048], m' in [0, 3072)
                for dot in range(4):
                    nc.vector.tensor_copy(
                        rh2T[:, dot * 3072 : dot * 3072 + 1025],
                        rhT[:, dot * T + 1023 : dot * T + 2048])
                    nc.vector.tensor_copy(
                        rh2T[:, dot * 3072 + 1025 : dot * 3072 + 3072],
                        rhT[:, dot * T : dot * T + 2047])

                # vh (untransposed): per key tile tt, psum [128 keys, 512 dh]
                wsb = load_w(jw, Wv)
                for tg in range(2):
                    vps = {tl: jps.tile([P, 512], F32, tag="proj", name="vhps")
                           for tl in range(8)}
                    for kd in range(8):
                        vsb = jp.tile([P, 1024], F32R, tag="xstage")
                        nc.sync.dma_start(
                            out=vsb[:],
                            in_=vT[kd * P : (kd + 1) * P,
                                   tg * 1024 : (tg + 1) * 1024],
                        )
                        for tl in range(8):
                            nc.tensor.matmul(
                                vps[tl][:],
                                vsb[:, tl * P : (tl + 1) * P],
                                wsb[:, kd * DH : (kd + 1) * DH],
                                start=(kd == 0),
                                stop=(kd == 7),
                            )
                    for tl in range(8):
                        tt = tg * 8 + tl
                        ps = vps[tl]
                        base = tt * (HC * 65)
                        dst = bass.AP(vh16.tensor, vh16.offset + base,
                                      [[vh16.tensor.shape[1], P], [65, HC], [1, DK]])
                        nc.vector.tensor_copy(dst, ps[:].rearrange("p (h c) -> p h c", h=HC))
                        ones = bass.AP(vh16.tensor, vh16.offset + base + DK,
                                       [[vh16.tensor.shape[1], P], [65, HC]])
                        nc.vector.memset(ones, 1.0)

            # ---------------- attention ----------------
            with (
                tc.tile_pool(name="att_m", bufs=2) as mp,
                tc.tile_pool(name="att_sc", bufs=2) as scp,
                tc.tile_pool(name="att_att", bufs=4) as atp,
                tc.tile_pool(name="att_tr", bufs=3) as trp,
                tc.tile_pool(name="dram", bufs=4, space="DRAM") as dp,
                tc.tile_pool(name="ps_m", bufs=2, space="PSUM") as psm,
                tc.tile_pool(name="ps_ac", bufs=4, space="PSUM") as psac,
                tc.tile_pool(name="ps_o", bufs=2, space="PSUM") as pso,
                tc.tile_pool(name="smalls", bufs=4) as smp,
            ):
                for hp in range(4):
                    att_tiles = {}
                    for qi in range(NQT):
                        KQ = kq_of(qi)
                        W = KQ + 127
                        for h in range(2):
                            pr = slice(h * DK, (h + 1) * DK)
                            # position-score matrix M [128, W]
                            msb = mp.tile([P, WMAX], F16, tag="msb")
                            nwc = (W + 511) // 512
                            for wc in range(nwc):
                                nw = min(512, W - wc * 512)
                                mps = psm.tile([P, 512], F32, tag="mps")
                                nc.tensor.matmul(
                                    mps[:, :nw],
                                    qvT[pr, hp * S + qi * P : hp * S + (qi + 1) * P],
                                    rh2T[pr, hp * 3072 + qi * P + wc * 512 :
                                         hp * 3072 + qi * P + wc * 512 + nw],
                                    start=True, stop=True,
                                )
                                nc.vector.tensor_copy(
                                    msb[:, wc * 512 : wc * 512 + nw], mps[:, :nw])
                            # shear via HBM: write rows stride W, read stride W+1
                            mdr = dp.tile([SLOT], F16, tag="mscr")
                            nc.sync.dma_start(
                                out=bass.AP(mdr.tensor, mdr.offset, [[W, P], [1, W]]),
                                in_=msb[:, :W],
                            )
                            bd = mp.tile([P, T], F16, tag="bd")
                            nc.sync.dma_start(
                                out=bd[:, :KQ],
                                in_=bass.AP(mdr.tensor, mdr.offset,
                                            [[W + 1, P], [1, KQ]]),
                            )
                            # content scores + combine + exp
                            att = atp.tile([P, T], F16, tag="att")
                            att_tiles[(h, qi)] = att
                            ssb = scp.tile([P, T], F32, tag="ssb")
                            nkc = (KQ + 511) // 512
                            for kc in range(nkc):
                                nk = min(512, KQ - kc * 512)
                                acps = psac.tile([P, 512], F32, tag="acps")
                                nc.tensor.matmul(
                                    acps[:, :nk],
                                    quT[pr, hp * S + qi * P : hp * S + (qi + 1) * P],
                                    khT[pr, hp * T + kc * 512 : hp * T + kc * 512 + nk],
                                    start=True, stop=True,
                                )
                                nc.vector.tensor_tensor(
                                    ssb[:, kc * 512 : kc * 512 + nk],
                                    acps[:, :nk],
                                    bd[:, kc * 512 : kc * 512 + nk],
                                    OP.add,
                                )
                            # causal boundary: block kj = qi+8 is tril
                            nc.vector.copy_predicated(
                                ssb[:, KQ - P : KQ], atril_sb[:], negbig_sb[:])
                            sums = smp.tile([P, 1], F32, tag="sums")
                            nc.scalar.activation(
                                att[:, :KQ], ssb[:, :KQ], AF.Exp,
                                bias=expb_sb[:], scale=0.125,
                                accum_out=sums[:])
                            recip_q = smp.tile([P, 1], F32, tag="recipq")
                            nc.vector.reciprocal(recip_q[:], sums[:])
                            nc.vector.tensor_scalar(
                                att[:, :KQ], att[:, :KQ], recip_q[:], None,
                                OP.mult)

                        if qi % 2 == 1:
                            # AV for query tiles (qi-1, qi), all valid keys
                            KQ0, KQ1 = kq_of(qi - 1), kq_of(qi)
                            njt = KQ1 // P
                            for h in range(2):
                                a0 = att_tiles.pop((h, qi - 1))
                                a1 = att_tiles.pop((h, qi))
                                nc.vector.memset(a0[:, KQ0:KQ1], 0.0)
                                ops = pso.tile([P, 256], F32, tag="ops")
                                for jt in range(njt):
                                    atr = trp.tile([P, 256], F16, tag="atr")
                                    nc.sync.dma_start_transpose(
                                        out=atr[:, 0:P],
                                        in_=a0[:, jt * P : (jt + 1) * P])
                                    nc.sync.dma_start_transpose(
                                        out=atr[:, P : 2 * P],
                                        in_=a1[:, jt * P : (jt + 1) * P])
                                    nc.tensor.matmul(
                                        ops[:65, :],
                                        vh16[:, jt * (HC * 65) + (hp * 2 + h) * 65 :
                                             jt * (HC * 65) + (hp * 2 + h) * 65 + 65],
                                        atr[:],
                                        start=(jt == 0), stop=(jt == njt - 1),
                                    )
                                qa = (qi - 1) // 2
                                nc.vector.tensor_copy(
                                    concatT[h * DK : (h + 1) * DK,
                                            hp * S + qa * 256 : hp * S + (qa + 1) * 256],
                                    ops[0:DK, :])

            # ---------------- output projection ----------------
            with (
                tc.tile_pool(name="outp", bufs=3) as op_,
                tc.tile_pool(name="outpsum", bufs=4, space="PSUM") as ops_,
            ):
                for it in range(8):
                    for oc in range(2):
                        ps = ops_.tile([P, 512], F32, tag="out")
                        for dt in range(4):
                            nc.tensor.matmul(
                                ps[:],
                                concatT[:, dt * S + it * P : dt * S + (it + 1) * P],
                                WoS[:, dt * D + oc * 512 : dt * D + (oc + 1) * 512],
                                start=(dt == 0), stop=(dt == 3),
                            )
                        osb = op_.tile([P, 512], F32, tag="osb")
                        nc.vector.tensor_copy(osb[:], ps[:])
                        nc.sync.dma_start(
                            out=outp[it * P : (it + 1) * P, oc * 512 : (oc + 1) * 512],
                            in_=osb[:])

    if split_waits:
        _split_multi_waits(nc)
    return nc


def prep_core_inputs(core, q, k, v, u, v_bias, Wq, Wk, Wv, Wr, Wo, R):
    b, hh = core // 2, core % 2
    sl = slice(hh * DH, (hh + 1) * DH)
    c = np.ascontiguousarray
    return {
        "qT": c(q[b].T),
        "kT": c(k[b].T),
        "vT": c(v[b].T),
        "RT": c(R.T),
        "Wq": c(Wq[sl, :].T),
        "Wk": c(Wk[sl, :].T),
        "Wv": c(Wv[sl, :].T),
        "Wr": c(Wr[sl, :].T),
        "Wo16": c(Wo[:, sl].T).astype(np.float16),
        "ub": c(u[0, hh * HC : (hh + 1) * HC, 0, :].reshape(4, P).T),
        "vb": c(v_bias[0, hh * HC : (hh + 1) * HC, 0, :].reshape(4, P).T),
        "atril": np.triu(np.ones((P, P), np.uint8), k=1),
    }


def combine_outputs(results):
    # results: list of 8 dicts with "out" [S, D]; partial sums per batch pair
    out = np.empty((4, S, D), np.float32)
    for b in range(4):
        out[b] = results[2 * b]["out"] + results[2 * b + 1]["out"]
    return out


_CACHED_NC = None
last_result = None  # BassKernelResults of the most recent run (for test harness)


def kernel(q, k, v, mask, u, v_bias, Wq, Wk, Wv, Wr, Wo, R):
    global _CACHED_NC, last_result
    from concourse.bass_utils import run_bass_kernel_spmd

    q, k, v = np.asarray(q), np.asarray(k), np.asarray(v)
    u, v_bias = np.asarray(u), np.asarray(v_bias)
    Wq, Wk, Wv, Wr, Wo, R = map(np.asarray, (Wq, Wk, Wv, Wr, Wo, R))

    # The kernel exploits the known TXL mask structure (j <= i + MEM).
    # Verify the passed mask matches; structural masking is baked in.
    m = np.asarray(mask)
    exp_mask = (np.arange(T)[None, :] <= np.arange(S)[:, None] + 1024)
    assert m.shape == (4, S, T) and bool((m == exp_mask[None]).all()), \
        "kernel compiled for the TXL causal mask (j <= i + MEM)"

    if _CACHED_NC is None:
        _CACHED_NC = build_nc()

    in_maps = [prep_core_inputs(c, q, k, v, u, v_bias, Wq, Wk, Wv, Wr, Wo, R)
               for c in range(8)]
    trace = bool(os.environ.get("TXL_TRACE"))
    kwargs = {}
    if trace:
        kwargs = {"trace": True, "tmpdir": os.environ.get("TXL_TRACE_DIR")}
    last_result = run_bass_kernel_spmd(_CACHED_NC, in_maps, list(range(8)), **kwargs)
    return combine_outputs(last_result.results)



# revision 5
# speedup vs baseline: 2.3065x; 2.3065x over previous
"""Transformer-XL relative multi-head attention, 8-way sharded on Trainium2.

Self-contained harness entry: kernel(**inputs) -> np.ndarray [4, 1024, 1024].

Sharding: core c handles batch b = c//2 and head-half hh = c%2 (8 of 16
heads). Each core computes a partial output (its heads' contribution
through Wo); the host unshard sums the two partials per batch (row-parallel
tensor parallelism for the output projection).

v2 pipeline per (head-pair hp, query tile qi, head h):
  - position matmul M [128, W] (K=64, two heads packed in PE row groups)
  - M psum evacuated fp16 (DVE/ACT split), HBM shear roundtrip -> bd
  - causal tail mask added onto bd via GPSIMD
  - content matmul + PE identity-add of bd (4 concurrent 32x32 diagonal
    tiles) accumulate in PSUM; ScalarE exp from PSUM with accum_out sums
  - DVE normalize (fp16 4x), chunked single-instruction DMA transpose
  - AV matmul with both heads packed via PE column groups
"""

import os
import sys

sys.path.insert(0, "/opt/trn_rl_repo")

import numpy as np


import concourse.bass as bass
import concourse.mybir as mybir
from concourse.tile import TileContext, ScopedClock

F32 = mybir.dt.float32
F16 = mybir.dt.float16
AF = mybir.ActivationFunctionType
OP = mybir.AluOpType

S, T, D, HC, DK, P = 1024, 2048, 1024, 8, 64, 128
DH = HC * DK  # 512, head-slice width per core
NQT = S // P  # 8 query tiles
WMAX = 2048 + 127  # max W (qi=7)
SLOT = P * (WMAX + 1) + 64  # dram scratch slot elements
EXP_BIAS = -7.0
NEG_BIG = -60000.0


def _patched_drain_and_barrier(self, tick_clock, wait_clock):
    # The walrus build in this container caps sync-waits per instruction;
    # Tile's stock tail drain carries one wait per live proc. Emit one SP nop
    # per wait instead, then the drain.
    dummy = mybir.InstNoOp(name="drain-wait-probe", ins=[], outs=[])
    dummy.engine = mybir.EngineType.SP
    wait_clock.add_sem_waits(dummy, ScopedClock({None: tick_clock.global_clock}))
    waits = []
    if dummy.sync_info is not None and dummy.sync_info.on_wait:
        waits = [(w.ant_name, w.wait_value) for w in dummy.sync_info.on_wait]
    assert self.sems is not None
    name2sem = {h.name: h for h in self.sems.allocated().values()}
    for name, val in waits:
        self.nc.sync.nop().wait_op(name2sem[name], val, "sem-ge")
    self.nc.sync.drain()
    self.nc.all_engine_barrier()
    popped = self.nc._tile_sem_poison_stack.pop()
    assert popped is self._sem_poison
    self.nc.clear_and_free_semaphores(list(self.sems.allocated().values()))
    self.nc.all_engine_barrier()


TileContext._drain_and_barrier = _patched_drain_and_barrier


def _split_multi_waits(nc, max_waits=1):
    """Walrus in this container rejects instructions carrying more than a
    couple of sync waits. Hoist extras onto same-engine NoOps just before
    the instruction (sequential on the engine, so semantics unchanged)."""
    for f in nc.m.functions:
        for bb in f.blocks:
            out = []
            changed = False
            for inst in bb.instructions:
                si = inst.sync_info
                if si is not None and si.on_wait and len(si.on_wait) > max_waits:
                    waits = list(si.on_wait)
                    for j, w in enumerate(waits[:-max_waits]):
                        nop = mybir.InstNoOp(
                            name=f"{inst.name}-wsplit{j}", ins=[], outs=[])
                        nop.engine = inst.engine
                        nop.sync_info = mybir.SyncInfo(on_wait=[w], on_update=[])
                        out.append(nop)
                    inst.sync_info = mybir.SyncInfo(
                        on_wait=waits[-max_waits:],
                        on_update=list(si.on_update))
                    changed = True
                out.append(inst)
            if changed:
                bb.instructions = out


def kq_of(qi):  # valid key count for query tile qi (keys j <= i + 1024)
    return (qi + 9) * P


def build_nc(split_waits=True):
    nc = bass.Bass(target_bir_lowering=True)

    qT = nc.declare_dram_parameter("qT", [D, S], F16, isOutput=False)
    kT = nc.declare_dram_parameter("kT", [D, T], F16, isOutput=False)
    vT = nc.declare_dram_parameter("vT", [D, T], F16, isOutput=False)
    RT = nc.declare_dram_parameter("RT", [D, T], F16, isOutput=False)
    Wq = nc.declare_dram_parameter("Wq", [D, DH], F16, isOutput=False)
    Wk = nc.declare_dram_parameter("Wk", [D, DH], F16, isOutput=False)
    Wv = nc.declare_dram_parameter("Wv", [D, DH], F16, isOutput=False)
    Wr = nc.declare_dram_parameter("Wr", [D, DH], F16, isOutput=False)
    Wo16 = nc.declare_dram_parameter("Wo16", [DH, D], F16, isOutput=False)
    ub = nc.declare_dram_parameter("ub", [P, 4], F32, isOutput=False)
    vb = nc.declare_dram_parameter("vb", [P, 4], F32, isOutput=False)
    masktail = nc.declare_dram_parameter("masktail", [P, P], F16, isOutput=False)
    i32x4 = nc.declare_dram_parameter("i32x4", [P, 32], F16, isOutput=False)
    outp = nc.declare_dram_parameter("out", [S, D], F32, isOutput=True)

    with TileContext(nc) as tc:
        with (
            tc.tile_pool(name="persist", bufs=1) as pp,
            tc.tile_pool(name="consts", bufs=1) as cp,
        ):
            # persistent fp16 tensors (partition = dk within head-pair tile)
            quT = pp.tile([P, 4 * S], F16)      # (qh+u).T   blocks hp
            qvT = pp.tile([P, 4 * S], F16)      # (qh+v).T
            khT = pp.tile([P, 4 * T], F16)
            rh2T = pp.tile([P, 4 * 3072], F16)
            vh16 = pp.tile([P, 16 * (HC * DK)], F16)  # key tile x 8 heads x 64
            concatT = pp.tile([P, 4 * S], F16)
            WoS = pp.tile([P, 4 * D], F16)

            ub_sb = cp.tile([P, 4], F32)
            vb_sb = cp.tile([P, 4], F32)
            mt_sb = cp.tile([P, P], F16)
            i32_sb = cp.tile([P, 32], F16)
            expb_sb = cp.tile([P, 1], F32)
            nc.vector.memset(expb_sb[:], EXP_BIAS)

            nc.sync.dma_start(out=ub_sb[:], in_=ub[:])
            nc.sync.dma_start(out=vb_sb[:], in_=vb[:])
            nc.sync.dma_start(out=mt_sb[:], in_=masktail[:])
            nc.sync.dma_start(out=i32_sb[:], in_=i32x4[:])
            # WoS layout [128, dt*1024 + o] <- Wo16[(dt p), o]
            for dt_ in range(4):
                nc.scalar.dma_start(
                    out=WoS[:, dt_ * D : (dt_ + 1) * D],
                    in_=Wo16[dt_ * P : (dt_ + 1) * P, :],
                )

            # ---------------- projections ----------------
            def load_w(pool, wparam):
                wsb = pool.tile([P, 8 * DH], F16, tag="wsb")
                for kd in range(8):
                    nc.scalar.dma_start(
                        out=wsb[:, kd * DH : (kd + 1) * DH],
                        in_=wparam[kd * P : (kd + 1) * P, :],
                    )
                return wsb

            # qhT-style projection: out[512, ncols] = W_s @ xT, evacuated by fn
            def proj_T(pool, psum, wsb, xparam, ncols, evac):
                nth = ncols // 1024
                for th in range(nth):
                    psums = {k: psum.tile([P, 512], F32, tag="proj", name="projps")
                             for k in [(d, t2) for d in range(4) for t2 in range(2)]}
                    for kd in range(8):
                        xsb = pool.tile([P, 1024], F16, tag="xstage")
                        nc.scalar.dma_start(
                            out=xsb[:],
                            in_=xparam[kd * P : (kd + 1) * P,
                                       th * 1024 : (th + 1) * 1024],
                        )
                        for dot in range(4):
                            for tc2 in range(2):
                                nc.tensor.matmul(
                                    psums[(dot, tc2)][:],
                                    wsb[:, kd * DH + dot * P : kd * DH + (dot + 1) * P],
                                    xsb[:, tc2 * 512 : (tc2 + 1) * 512],
                                    start=(kd == 0),
                                    stop=(kd == 7),
                                )
                    for dot in range(4):
                        for tc2 in range(2):
                            evac(psums[(dot, tc2)], dot, th * 1024 + tc2 * 512)

            with (
                tc.tile_pool(name="projp", bufs=3) as jp,
                tc.tile_pool(name="projw", bufs=2) as jw,
                tc.tile_pool(name="rhtmp", bufs=1) as jr,
                tc.tile_pool(name="projpsum", bufs=8, space="PSUM") as jps,
            ):
                wsb = load_w(jw, Wq)

                def evac_q(ps, dot, col):
                    nc.vector.tensor_scalar(
                        quT[:, dot * S + col : dot * S + col + 512], ps[:],
                        ub_sb[:, dot : dot + 1], None, OP.add)
                    nc.vector.tensor_scalar(
                        qvT[:, dot * S + col : dot * S + col + 512], ps[:],
                        vb_sb[:, dot : dot + 1], None, OP.add)

                proj_T(jp, jps, wsb, qT, S, evac_q)

                wsb = load_w(jw, Wk)

                def evac_k(ps, dot, col):
                    nc.scalar.copy(
                        khT[:, dot * T + col : dot * T + col + 512], ps[:])

                proj_T(jp, jps, wsb, kT, T, evac_k)

                rhT = jr.tile([P, 4 * T], F16, tag="rhT")
                wsb = load_w(jw, Wr)

                def evac_r(ps, dot, col):
                    nc.vector.tensor_copy(
                        rhT[:, dot * T + col : dot * T + col + 512], ps[:])

                proj_T(jp, jps, wsb, RT, T, evac_r)

                # rh2T[:, m'] = rhT[:, (m' + 1023) % 2048], m' in [0, 3072)
                for dot in range(4):
                    nc.vector.tensor_copy(
                        rh2T[:, dot * 3072 : dot * 3072 + 1025],
                        rhT[:, dot * T + 1023 : dot * T + 2048])
                    nc.vector.tensor_copy(
                        rh2T[:, dot * 3072 + 1025 : dot * 3072 + 3072],
                        rhT[:, dot * T : dot * T + 2047])

                # vh (untransposed): per key tile tt, psum [128 keys, 512 dh]
                wsb = load_w(jw, Wv)
                for tg in range(2):
                    vps = {tl: jps.tile([P, 512], F32, tag="proj", name="vhps")
                           for tl in range(8)}
                    for kd in range(8):
                        vsb = jp.tile([P, 1024], F16, tag="xstage")
                        nc.scalar.dma_start(
                            out=vsb[:],
                            in_=vT[kd * P : (kd + 1) * P,
                                   tg * 1024 : (tg + 1) * 1024],
                        )
                        for tl in range(8):
                            nc.tensor.matmul(
                                vps[tl][:],
                                vsb[:, tl * P : (tl + 1) * P],
                                wsb[:, kd * DH : (kd + 1) * DH],
                                start=(kd == 0),
                                stop=(kd == 7),
                            )
                    for tl in range(8):
                        tt = tg * 8 + tl
                        nc.scalar.copy(
                            vh16[:, tt * DH : (tt + 1) * DH], vps[tl][:])

            # ---------------- attention ----------------
            with (
                tc.tile_pool(name="att_m", bufs=2) as mp,
                tc.tile_pool(name="att_bd", bufs=3) as bp,
                tc.tile_pool(name="att_att", bufs=4) as atp,
                tc.tile_pool(name="att_tr", bufs=2) as trp,
                tc.tile_pool(name="dram", bufs=6, space="DRAM") as dp,
                tc.tile_pool(name="ps_m", bufs=2, space="PSUM") as psm,
                tc.tile_pool(name="ps_ac", bufs=2, space="PSUM") as psac,
                tc.tile_pool(name="ps_o", bufs=2, space="PSUM") as pso,
                tc.tile_pool(name="smalls", bufs=6) as smp,
            ):
                for hp in range(4):
                    atr_tiles = {}
                    kq_pair = {}
                    for qi in range(NQT):
                        KQ = kq_of(qi)
                        W = KQ + 127
                        njt = KQ // P
                        if qi % 2 == 0:
                            # allocate the pair's transpose tiles (a0|a1
                            # interleaved per 128-key chunk, 256 cols each)
                            KQ1 = kq_of(qi + 1)
                            for h in range(2):
                                atr_tiles[h] = trp.tile(
                                    [P, 256 * (KQ1 // P)], F16, tag="atr",
                                    name=f"atr_{hp}_{qi}_{h}")
                            kq_pair[0] = KQ
                        for h in range(2):
                            pr = slice(h * DK, (h + 1) * DK)
                            # position-score matrix M [128, W] (K=64 matmuls;
                            # h=0 rows 0-63, h=1 rows 64-127 pack in PE)
                            msb = mp.tile([P, WMAX], F16, tag="msb")
                            nwc = (W + 511) // 512
                            for wc in range(nwc):
                                nw = min(512, W - wc * 512)
                                mps = psm.tile([P, 512], F32, tag="mps")
                                nc.tensor.matmul(
                                    mps[:, :nw],
                                    qvT[pr, hp * S + qi * P : hp * S + (qi + 1) * P],
                                    rh2T[pr, hp * 3072 + qi * P + wc * 512 :
                                         hp * 3072 + qi * P + wc * 512 + nw],
                                    start=True, stop=True,
                                )
                                # psum evac split: mostly DVE, some ACT
                                if wc == nwc - 1:
                                    nc.scalar.copy(
                                        msb[:, wc * 512 : wc * 512 + nw],
                                        mps[:, :nw])
                                else:
                                    nc.vector.tensor_copy(
                                        msb[:, wc * 512 : wc * 512 + nw],
                                        mps[:, :nw])
                            # shear via HBM: write rows stride W, read stride W+1
                            mdr = dp.tile([SLOT], F16, tag="mscr")
                            nc.sync.dma_start(
                                out=bass.AP(mdr.tensor, mdr.offset, [[W, P], [1, W]]),
                                in_=msb[:, :W],
                            )
                            bd = bp.tile([P, T], F16, tag="bd")
                            nc.sync.dma_start(
                                out=bd[:, :KQ],
                                in_=bass.AP(mdr.tensor, mdr.offset,
                                            [[W + 1, P], [1, KQ]]),
                            )
                            # causal tail: add -big upper-triangle onto bd
                            nc.gpsimd.tensor_tensor(
                                bd[:, KQ - P : KQ], bd[:, KQ - P : KQ],
                                mt_sb[:], OP.add)

                            # content scores + id-add + exp (1024-col chunks)
                            att = atp.tile([P, T], F16, tag="att")
                            nkc2 = (KQ + 1023) // 1024
                            sumsp = smp.tile([P, 4], F32, tag="sumsp")
                            for kc in range(nkc2):
                                w2 = min(1024, KQ - kc * 1024)
                                acps = psac.tile([P, 1024], F32, tag="acps")
                                for sc in range((w2 + 511) // 512):
                                    off = kc * 1024 + sc * 512
                                    nk = min(512, KQ - off)
                                    so = sc * 512
                                    nc.tensor.matmul(
                                        acps[:, so : so + nk],
                                        quT[pr, hp * S + qi * P : hp * S + (qi + 1) * P],
                                        khT[pr, hp * T + off : hp * T + off + nk],
                                        start=True, stop=False,
                                    )
                                    for d4 in range(4):
                                        rp = slice(32 * d4, 32 * d4 + 32)
                                        nc.tensor.matmul(
                                            acps[rp, so : so + nk],
                                            i32_sb[rp, :],
                                            bd[rp, off : off + nk],
                                            start=False, stop=(d4 == 3),
                                            tile_position=(32 * d4, 32 * d4),
                                        )
                                nc.scalar.activation(
                                    att[:, kc * 1024 : kc * 1024 + w2],
                                    acps[:, :w2], AF.Exp,
                                    bias=expb_sb[:], scale=0.125,
                                    accum_out=sumsp[:, kc : kc + 1])
                            # merge partial sums (KQ > 1024 always => nkc2 == 2)
                            sums = smp.tile([P, 1], F32, tag="sums")
                            scr = smp.tile([P, 4], F32, tag="sumscratch")
                            nc.scalar.activation(
                                scr[:, :nkc2], sumsp[:, :nkc2], AF.Copy,
                                accum_out=sums[:])
                            recip_q = smp.tile([P, 1], F32, tag="recipq")
                            nc.vector.reciprocal(recip_q[:], sums[:])
                            nc.vector.tensor_scalar(
                                att[:, :KQ], att[:, :KQ], recip_q[:], None,
                                OP.mult)
                            # chunked transpose into the pair slot
                            atr = atr_tiles[h]
                            nc.scalar.dma_start_transpose(
                                out=bass.AP(
                                    atr.tensor,
                                    atr.offset + (qi % 2) * P,
                                    [[atr.tensor.shape[1], P], [256, njt], [1, P]],
                                ),
                                in_=att[:, :KQ],
                            )

                        if qi % 2 == 1:
                            # AV for the pair, both heads packed via col groups
                            KQ0, KQ1 = kq_pair[0], KQ
                            njt1 = KQ1 // P
                            ops = pso.tile([P, 256], F32, tag="ops")
                            for h in range(2):
                                # zero-fill a0's missing key chunks
                                atr = atr_tiles[h]
                                for jt in range(KQ0 // P, njt1):
                                    nc.vector.memset(
                                        atr[:, jt * 256 : jt * 256 + P], 0.0)
                            for jt in range(njt1):
                                for h in range(2):
                                    atr = atr_tiles[h]
                                    nc.tensor.matmul(
                                        ops[h * DK : (h + 1) * DK, :],
                                        vh16[:, jt * DH + (hp * 2 + h) * DK :
                                             jt * DH + (hp * 2 + h) * DK + DK],
                                        atr[:, jt * 256 : (jt + 1) * 256],
                                        start=(jt == 0), stop=(jt == njt1 - 1),
                                        tile_position=(0, h * DK),
                                    )
                            qa = (qi - 1) // 2
                            nc.vector.tensor_copy(
                                concatT[:, hp * S + qa * 256 : hp * S + (qa + 1) * 256],
                                ops[:])

            # ---------------- output projection ----------------
            with (
                tc.tile_pool(name="outp", bufs=3) as op_,
                tc.tile_pool(name="outpsum", bufs=4, space="PSUM") as ops_,
            ):
                for it in range(8):
                    for oc in range(2):
                        ps = ops_.tile([P, 512], F32, tag="out")
                        for dt in range(4):
                            nc.tensor.matmul(
                                ps[:],
                                concatT[:, dt * S + it * P : dt * S + (it + 1) * P],
                                WoS[:, dt * D + oc * 512 : dt * D + (oc + 1) * 512],
                                start=(dt == 0), stop=(dt == 3),
                            )
                        osb = op_.tile([P, 512], F32, tag="osb")
                        nc.vector.tensor_copy(osb[:], ps[:])
                        nc.sync.dma_start(
                            out=outp[it * P : (it + 1) * P, oc * 512 : (oc + 1) * 512],
                            in_=osb[:])

    if split_waits:
        _split_multi_waits(nc)
    return nc


def prep_core_inputs(core, q, k, v, u, v_bias, Wq, Wk, Wv, Wr, Wo, R):
    b, hh = core // 2, core % 2
    sl = slice(hh * DH, (hh + 1) * DH)
    c = np.ascontiguousarray
    f16 = np.float16
    i32x4 = np.zeros((P, 32), f16)
    for d4 in range(4):
        i32x4[32 * d4 : 32 * d4 + 32] = np.eye(32, dtype=f16)
    ii = np.arange(P)
    masktail = np.where(ii[None, :] <= ii[:, None], 0.0, NEG_BIG).astype(f16)
    return {
        "qT": c(q[b].T).astype(f16),
        "kT": c(k[b].T).astype(f16),
        "vT": c(v[b].T).astype(f16),
        "RT": c(R.T).astype(f16),
        "Wq": c(Wq[sl, :].T).astype(f16),
        "Wk": c(Wk[sl, :].T).astype(f16),
        "Wv": c(Wv[sl, :].T).astype(f16),
        "Wr": c(Wr[sl, :].T).astype(f16),
        "Wo16": c(Wo[:, sl].T).astype(f16),
        "ub": c(u[0, hh * HC : (hh + 1) * HC, 0, :].reshape(4, P).T),
        "vb": c(v_bias[0, hh * HC : (hh + 1) * HC, 0, :].reshape(4, P).T),
        "masktail": masktail,
        "i32x4": i32x4,
    }


def combine_outputs(results):
    # results: list of 8 dicts with "out" [S, D]; partial sums per batch pair
    out = np.empty((4, S, D), np.float32)
    for b in range(4):
        out[b] = results[2 * b]["out"] + results[2 * b + 1]["out"]
    return out


_CACHED_NC = None
last_result = None  # BassKernelResults of the most recent run (for test harness)


def kernel(q, k, v, mask, u, v_bias, Wq, Wk, Wv, Wr, Wo, R):
    global _CACHED_NC, last_result
    from concourse.bass_utils import run_bass_kernel_spmd

    q, k, v = np.asarray(q), np.asarray(k), np.asarray(v)
    u, v_bias = np.asarray(u), np.asarray(v_bias)
    Wq, Wk, Wv, Wr, Wo, R = map(np.asarray, (Wq, Wk, Wv, Wr, Wo, R))

    # The kernel exploits the known TXL mask structure (j <= i + MEM).
    # Verify the passed mask matches; structural masking is baked in.
    m = np.asarray(mask)
    exp_mask = (np.arange(T)[None, :] <= np.arange(S)[:, None] + 1024)
    assert m.shape == (4, S, T) and bool((m == exp_mask[None]).all()), \
        "kernel compiled for the TXL causal mask (j <= i + MEM)"

    if _CACHED_NC is None:
        _CACHED_NC = build_nc()

    in_maps = [prep_core_inputs(c, q, k, v, u, v_bias, Wq, Wk, Wv, Wr, Wo, R)
               for c in range(8)]
    trace = bool(os.environ.get("TXL_TRACE"))
    kwargs = {}
    if trace:
        kwargs = {"trace": True, "tmpdir": os.environ.get("TXL_TRACE_DIR")}
    last_result = run_bass_kernel_spmd(_CACHED_NC, in_maps, list(range(8)), **kwargs)
    return combine_outputs(last_result.results)


# revision 11
# speedup vs baseline: 2.5964x; 1.1257x over previous
"""Transformer-XL relative multi-head attention, 8-way sharded on Trainium2.

Self-contained harness entry: kernel(**inputs) -> np.ndarray [4, 1024, 1024].

Sharding: core c handles batch b = c//2 and head-half hh = c%2 (8 of 16
heads). Each core computes a partial output (its heads' contribution
through Wo); the host unshard sums the two partials per batch (row-parallel
tensor parallelism for the output projection).

v2 pipeline per (head-pair hp, query tile qi, head h):
  - position matmul M [128, W] (K=64, two heads packed in PE row groups)
  - M psum evacuated fp16 (DVE/ACT split), HBM shear roundtrip -> bd
  - causal tail mask added onto bd via GPSIMD
  - content matmul + PE identity-add of bd (4 concurrent 32x32 diagonal
    tiles) accumulate in PSUM; ScalarE exp from PSUM with accum_out sums
  - DVE normalize (fp16 4x), chunked single-instruction DMA transpose
  - AV matmul with both heads packed via PE column groups
"""

import os
import sys

sys.path.insert(0, "/opt/trn_rl_repo")

import numpy as np


import concourse.bass as bass
import concourse.mybir as mybir
from concourse.tile import TileContext, ScopedClock

F32 = mybir.dt.float32
F16 = mybir.dt.float16
AF = mybir.ActivationFunctionType
OP = mybir.AluOpType

S, T, D, HC, DK, P = 1024, 2048, 1024, 8, 64, 128
DH = HC * DK  # 512, head-slice width per core
NQT = S // P  # 8 query tiles
WMAX = 2048 + 127  # max W (qi=7)
SLOT = P * (WMAX + 1) + 64  # dram scratch slot elements
EXP_BIAS = -7.0
NEG_BIG = -60000.0


def _patched_drain_and_barrier(self, tick_clock, wait_clock):
    # The walrus build in this container caps sync-waits per instruction;
    # Tile's stock tail drain carries one wait per live proc. Emit one SP nop
    # per wait instead, then the drain.
    dummy = mybir.InstNoOp(name="drain-wait-probe", ins=[], outs=[])
    dummy.engine = mybir.EngineType.SP
    wait_clock.add_sem_waits(dummy, ScopedClock({None: tick_clock.global_clock}))
    waits = []
    if dummy.sync_info is not None and dummy.sync_info.on_wait:
        waits = [(w.ant_name, w.wait_value) for w in dummy.sync_info.on_wait]
    assert self.sems is not None
    name2sem = {h.name: h for h in self.sems.allocated().values()}
    for name, val in waits:
        self.nc.sync.nop().wait_op(name2sem[name], val, "sem-ge")
    self.nc.sync.drain()
    self.nc.all_engine_barrier()
    popped = self.nc._tile_sem_poison_stack.pop()
    assert popped is self._sem_poison
    self.nc.clear_and_free_semaphores(list(self.sems.allocated().values()))
    self.nc.all_engine_barrier()


TileContext._drain_and_barrier = _patched_drain_and_barrier


def _split_multi_waits(nc, max_waits=1):
    """Walrus in this container rejects instructions carrying more than a
    couple of sync waits. Hoist extras onto same-engine NoOps just before
    the instruction (sequential on the engine, so semantics unchanged)."""
    for f in nc.m.functions:
        for bb in f.blocks:
            out = []
            changed = False
            for inst in bb.instructions:
                si = inst.sync_info
                if si is not None and si.on_wait and len(si.on_wait) > max_waits:
                    waits = list(si.on_wait)
                    for j, w in enumerate(waits[:-max_waits]):
                        nop = mybir.InstNoOp(
                            name=f"{inst.name}-wsplit{j}", ins=[], outs=[])
                        nop.engine = inst.engine
                        nop.sync_info = mybir.SyncInfo(on_wait=[w], on_update=[])
                        out.append(nop)
                    inst.sync_info = mybir.SyncInfo(
                        on_wait=waits[-max_waits:],
                        on_update=list(si.on_update))
                    changed = True
                out.append(inst)
            if changed:
                bb.instructions = out


def kq_of(qi):  # valid key count for query tile qi (keys j <= i + 1024)
    return (qi + 9) * P


def build_nc(split_waits=True):
    nc = bass.Bass(target_bir_lowering=True)

    qT = nc.declare_dram_parameter("qT", [D, S], F16, isOutput=False)
    kT = nc.declare_dram_parameter("kT", [D, T], F16, isOutput=False)
    vT = nc.declare_dram_parameter("vT", [D, T], F16, isOutput=False)
    RT = nc.declare_dram_parameter("RT", [D, T], F16, isOutput=False)
    Wq = nc.declare_dram_parameter("Wq", [D, DH], F16, isOutput=False)
    Wk = nc.declare_dram_parameter("Wk", [D, DH], F16, isOutput=False)
    Wv = nc.declare_dram_parameter("Wv", [D, DH], F16, isOutput=False)
    Wr = nc.declare_dram_parameter("Wr", [D, DH], F16, isOutput=False)
    Wo16 = nc.declare_dram_parameter("Wo16", [DH, D], F16, isOutput=False)
    ub = nc.declare_dram_parameter("ub", [P, 4], F32, isOutput=False)
    vb = nc.declare_dram_parameter("vb", [P, 4], F32, isOutput=False)
    masktail = nc.declare_dram_parameter("masktail", [P, P], F16, isOutput=False)
    i128 = nc.declare_dram_parameter("i128", [P, P], F16, isOutput=False)
    outp = nc.declare_dram_parameter("out", [S, D], F32, isOutput=True)

    with TileContext(nc) as tc:
        with (
            tc.tile_pool(name="persist", bufs=1) as pp,
            tc.tile_pool(name="consts", bufs=1) as cp,
        ):
            # persistent fp16 tensors (partition = dk within head-pair tile)
            quT = pp.tile([P, 4 * S], F16)      # (qh+u).T   blocks hp
            qvT = pp.tile([P, 4 * S], F16)      # (qh+v).T
            khT = pp.tile([P, 4 * T], F16)
            rh2T = pp.tile([P, 4 * 3072], F16)
            vh16 = pp.tile([P, 16 * (HC * DK)], F16)  # key tile x 8 heads x 64
            concatT = pp.tile([P, 4 * S], F16)
            WoS = pp.tile([P, 4 * D], F16)

            ub_sb = cp.tile([P, 4], F32)
            vb_sb = cp.tile([P, 4], F32)
            mt_sb = cp.tile([P, P], F16)
            i128_sb = cp.tile([P, P], F16)
            expb_sb = cp.tile([P, 1], F32)
            nc.vector.memset(expb_sb[:], EXP_BIAS)

            nc.sync.dma_start(out=ub_sb[:], in_=ub[:])
            nc.sync.dma_start(out=vb_sb[:], in_=vb[:])
            nc.sync.dma_start(out=mt_sb[:], in_=masktail[:])
            nc.sync.dma_start(out=i128_sb[:], in_=i128[:])
            # WoS layout [128, dt*1024 + o] <- Wo16[(dt p), o]
            for dt_ in range(4):
                nc.scalar.dma_start(
                    out=WoS[:, dt_ * D : (dt_ + 1) * D],
                    in_=Wo16[dt_ * P : (dt_ + 1) * P, :],
                )

            # ---------------- projections ----------------
            def load_w(pool, wparam):
                wsb = pool.tile([P, 8 * DH], F16, tag="wsb")
                for kd in range(8):
                    nc.scalar.dma_start(
                        out=wsb[:, kd * DH : (kd + 1) * DH],
                        in_=wparam[kd * P : (kd + 1) * P, :],
                    )
                return wsb

            # qhT-style projection: out[512, ncols] = W_s @ xT, evacuated by fn
            def proj_T(pool, psum, wsb, xparam, ncols, evac):
                nth = ncols // 1024
                for th in range(nth):
                    psums = {k: psum.tile([P, 512], F32, tag="proj", name="projps")
                             for k in [(d, t2) for d in range(4) for t2 in range(2)]}
                    for kd in range(8):
                        xsb = pool.tile([P, 1024], F16, tag="xstage")
                        nc.scalar.dma_start(
                            out=xsb[:],
                            in_=xparam[kd * P : (kd + 1) * P,
                                       th * 1024 : (th + 1) * 1024],
                        )
                        for dot in range(4):
                            for tc2 in range(2):
                                nc.tensor.matmul(
                                    psums[(dot, tc2)][:],
                                    wsb[:, kd * DH + dot * P : kd * DH + (dot + 1) * P],
                                    xsb[:, tc2 * 512 : (tc2 + 1) * 512],
                                    start=(kd == 0),
                                    stop=(kd == 7),
                                )
                    for dot in range(4):
                        for tc2 in range(2):
                            evac(psums[(dot, tc2)], dot, th * 1024 + tc2 * 512)

            with (
                tc.tile_pool(name="projp", bufs=3) as jp,
                tc.tile_pool(name="projw", bufs=2) as jw,
                tc.tile_pool(name="rhtmp", bufs=1) as jr,
                tc.tile_pool(name="projpsum", bufs=8, space="PSUM") as jps,
            ):
                wsb = load_w(jw, Wq)

                def evac_q(ps, dot, col):
                    nc.vector.tensor_scalar(
                        quT[:, dot * S + col : dot * S + col + 512], ps[:],
                        ub_sb[:, dot : dot + 1], None, OP.add)
                    nc.vector.tensor_scalar(
                        qvT[:, dot * S + col : dot * S + col + 512], ps[:],
                        vb_sb[:, dot : dot + 1], None, OP.add)

                proj_T(jp, jps, wsb, qT, S, evac_q)

                wsb = load_w(jw, Wk)

                def evac_k(ps, dot, col):
                    nc.scalar.copy(
                        khT[:, dot * T + col : dot * T + col + 512], ps[:])

                proj_T(jp, jps, wsb, kT, T, evac_k)

                rhT = jr.tile([P, 4 * T], F16, tag="rhT")
                wsb = load_w(jw, Wr)

                def evac_r(ps, dot, col):
                    nc.vector.tensor_copy(
                        rhT[:, dot * T + col : dot * T + col + 512], ps[:])

                proj_T(jp, jps, wsb, RT, T, evac_r)

                # rh2T[:, m'] = rhT[:, (m' + 1023) % 2048], m' in [0, 3072)
                for dot in range(4):
                    nc.vector.tensor_copy(
                        rh2T[:, dot * 3072 : dot * 3072 + 1025],
                        rhT[:, dot * T + 1023 : dot * T + 2048])
                    nc.vector.tensor_copy(
                        rh2T[:, dot * 3072 + 1025 : dot * 3072 + 3072],
                        rhT[:, dot * T : dot * T + 2047])

                # vh (untransposed): per key tile tt, psum [128 keys, 512 dh]
                wsb = load_w(jw, Wv)
                for tg in range(2):
                    vps = {tl: jps.tile([P, 512], F32, tag="proj", name="vhps")
                           for tl in range(8)}
                    for kd in range(8):
                        vsb = jp.tile([P, 1024], F16, tag="xstage")
                        nc.scalar.dma_start(
                            out=vsb[:],
                            in_=vT[kd * P : (kd + 1) * P,
                                   tg * 1024 : (tg + 1) * 1024],
                        )
                        for tl in range(8):
                            nc.tensor.matmul(
                                vps[tl][:],
                                vsb[:, tl * P : (tl + 1) * P],
                                wsb[:, kd * DH : (kd + 1) * DH],
                                start=(kd == 0),
                                stop=(kd == 7),
                            )
                    for tl in range(8):
                        tt = tg * 8 + tl
                        nc.scalar.copy(
                            vh16[:, tt * DH : (tt + 1) * DH], vps[tl][:])

            # ---------------- attention ----------------
            with (
                tc.tile_pool(name="att_m", bufs=2) as mp,
                tc.tile_pool(name="att_bd", bufs=3) as bp,
                tc.tile_pool(name="att_att", bufs=4) as atp,
                tc.tile_pool(name="att_tr", bufs=2) as trp,
                tc.tile_pool(name="dram", bufs=6, space="DRAM") as dp,
                tc.tile_pool(name="ps_m", bufs=2, space="PSUM") as psm,
                tc.tile_pool(name="ps_ac", bufs=2, space="PSUM") as psac,
                tc.tile_pool(name="ps_o", bufs=2, space="PSUM") as pso,
                tc.tile_pool(name="smalls", bufs=6) as smp,
            ):
                for hp in range(4):
                    atr_tiles = {}
                    kq_pair = {}
                    for qi in range(NQT):
                        KQ = kq_of(qi)
                        W = KQ + 127
                        njt = KQ // P
                        if qi % 2 == 0:
                            # allocate the pair's transpose tiles (a0|a1
                            # interleaved per 128-key chunk, 256 cols each)
                            KQ1 = kq_of(qi + 1)
                            for h in range(2):
                                atr_tiles[h] = trp.tile(
                                    [P, 256 * (KQ1 // P)], F16, tag="atr",
                                    name=f"atr_{hp}_{qi}_{h}")
                            kq_pair[0] = KQ
                        for h in range(2):
                            pr = slice(h * DK, (h + 1) * DK)
                            # position-score matrix M [128, W] (K=64 matmuls;
                            # h=0 rows 0-63, h=1 rows 64-127 pack in PE)
                            msb = mp.tile([P, WMAX], F16, tag="msb")
                            nwc = (W + 511) // 512
                            for wc in range(nwc):
                                nw = min(512, W - wc * 512)
                                mps = psm.tile([P, 512], F32, tag="mps")
                                nc.tensor.matmul(
                                    mps[:, :nw],
                                    qvT[pr, hp * S + qi * P : hp * S + (qi + 1) * P],
                                    rh2T[pr, hp * 3072 + qi * P + wc * 512 :
                                         hp * 3072 + qi * P + wc * 512 + nw],
                                    start=True, stop=True,
                                )
                                # psum evac split: mostly DVE, some ACT
                                if wc == nwc - 1:
                                    nc.scalar.copy(
                                        msb[:, wc * 512 : wc * 512 + nw],
                                        mps[:, :nw])
                                else:
                                    nc.vector.tensor_copy(
                                        msb[:, wc * 512 : wc * 512 + nw],
                                        mps[:, :nw])
                            # shear via HBM: write rows stride W, read stride W+1
                            mdr = dp.tile([SLOT], F16, tag="mscr")
                            nc.sync.dma_start(
                                out=bass.AP(mdr.tensor, mdr.offset, [[W, P], [1, W]]),
                                in_=msb[:, :W],
                            )
                            bd = bp.tile([P, T], F16, tag="bd")
                            nc.sync.dma_start(
                                out=bd[:, :KQ],
                                in_=bass.AP(mdr.tensor, mdr.offset,
                                            [[W + 1, P], [1, KQ]]),
                            )
                            # causal tail: add -big upper-triangle onto bd
                            nc.gpsimd.tensor_tensor(
                                bd[:, KQ - P : KQ], bd[:, KQ - P : KQ],
                                mt_sb[:], OP.add)

                            # content scores + id-add + exp (1024-col chunks)
                            att = atp.tile([P, T], F16, tag="att")
                            nkc2 = (KQ + 1023) // 1024
                            sumsp = smp.tile([P, 4], F32, tag="sumsp")
                            for kc in range(nkc2):
                                w2 = min(1024, KQ - kc * 1024)
                                acps = psac.tile([P, 1024], F32, tag="acps")
                                nsc = (w2 + 511) // 512
                                for sc in range(nsc):
                                    off = kc * 1024 + sc * 512
                                    nk = min(512, KQ - off)
                                    so = sc * 512
                                    nc.tensor.matmul(
                                        acps[:, so : so + nk],
                                        quT[pr, hp * S + qi * P : hp * S + (qi + 1) * P],
                                        khT[pr, hp * T + off : hp * T + off + nk],
                                        start=True, stop=False,
                                    )
                                for sc in range(nsc):
                                    off = kc * 1024 + sc * 512
                                    nk = min(512, KQ - off)
                                    so = sc * 512
                                    nc.tensor.matmul(
                                        acps[:, so : so + nk],
                                        i128_sb[:],
                                        bd[:, off : off + nk],
                                        start=False, stop=True,
                                    )
                                nc.scalar.activation(
                                    att[:, kc * 1024 : kc * 1024 + w2],
                                    acps[:, :w2], AF.Exp,
                                    bias=expb_sb[:], scale=0.125,
                                    accum_out=sumsp[:, kc : kc + 1])
                            # merge partial sums (KQ > 1024 always => nkc2 == 2)
                            sums = smp.tile([P, 1], F32, tag="sums")
                            scr = smp.tile([P, 4], F32, tag="sumscratch")
                            nc.scalar.activation(
                                scr[:, :nkc2], sumsp[:, :nkc2], AF.Copy,
                                accum_out=sums[:])
                            recip_q = smp.tile([P, 1], F32, tag="recipq")
                            nc.vector.reciprocal(recip_q[:], sums[:])
                            nc.vector.tensor_scalar(
                                att[:, :KQ], att[:, :KQ], recip_q[:], None,
                                OP.mult)
                            # chunked transpose into the pair slot
                            atr = atr_tiles[h]
                            nc.sync.dma_start_transpose(
                                out=bass.AP(
                                    atr.tensor,
                                    atr.offset + (qi % 2) * P,
                                    [[atr.tensor.shape[1], P], [256, njt], [1, P]],
                                ),
                                in_=att[:, :KQ],
                            )

                        if qi % 2 == 1:
                            # AV for the pair, both heads packed via col groups
                            KQ0, KQ1 = kq_pair[0], KQ
                            njt1 = KQ1 // P
                            ops = pso.tile([P, 256], F32, tag="ops")
                            for h in range(2):
                                # zero-fill a0's missing key chunks
                                atr = atr_tiles[h]
                                for jt in range(KQ0 // P, njt1):
                                    nc.vector.memset(
                                        atr[:, jt * 256 : jt * 256 + P], 0.0)
                            for jt in range(njt1):
                                for h in range(2):
                                    atr = atr_tiles[h]
                                    nc.tensor.matmul(
                                        ops[h * DK : (h + 1) * DK, :],
                                        vh16[:, jt * DH + (hp * 2 + h) * DK :
                                             jt * DH + (hp * 2 + h) * DK + DK],
                                        atr[:, jt * 256 : (jt + 1) * 256],
                                        start=(jt == 0), stop=(jt == njt1 - 1),
                                        tile_position=(0, h * DK),
                                    )
                            qa = (qi - 1) // 2
                            nc.vector.tensor_copy(
                                concatT[:, hp * S + qa * 256 : hp * S + (qa + 1) * 256],
                                ops[:])

            # ---------------- output projection ----------------
            with (
                tc.tile_pool(name="outp", bufs=3) as op_,
                tc.tile_pool(name="outpsum", bufs=4, space="PSUM") as ops_,
            ):
                for it in range(8):
                    for oc in range(2):
                        ps = ops_.tile([P, 512], F32, tag="out")
                        for dt in range(4):
                            nc.tensor.matmul(
                                ps[:],
                                concatT[:, dt * S + it * P : dt * S + (it + 1) * P],
                                WoS[:, dt * D + oc * 512 : dt * D + (oc + 1) * 512],
                                start=(dt == 0), stop=(dt == 3),
                            )
                        osb = op_.tile([P, 512], F32, tag="osb")
                        nc.vector.tensor_copy(osb[:], ps[:])
                        nc.sync.dma_start(
                            out=outp[it * P : (it + 1) * P, oc * 512 : (oc + 1) * 512],
                            in_=osb[:])

    if split_waits:
        _split_multi_waits(nc)
    return nc


def prep_core_inputs(core, q, k, v, u, v_bias, Wq, Wk, Wv, Wr, Wo, R):
    b, hh = core // 2, core % 2
    sl = slice(hh * DH, (hh + 1) * DH)
    c = np.ascontiguousarray
    f16 = np.float16
    ii = np.arange(P)
    masktail = np.where(ii[None, :] <= ii[:, None], 0.0, NEG_BIG).astype(f16)
    return {
        "qT": c(q[b].T).astype(f16),
        "kT": c(k[b].T).astype(f16),
        "vT": c(v[b].T).astype(f16),
        "RT": c(R.T).astype(f16),
        "Wq": c(Wq[sl, :].T).astype(f16),
        "Wk": c(Wk[sl, :].T).astype(f16),
        "Wv": c(Wv[sl, :].T).astype(f16),
        "Wr": c(Wr[sl, :].T).astype(f16),
        "Wo16": c(Wo[:, sl].T).astype(f16),
        "ub": c(u[0, hh * HC : (hh + 1) * HC, 0, :].reshape(4, P).T),
        "vb": c(v_bias[0, hh * HC : (hh + 1) * HC, 0, :].reshape(4, P).T),
        "masktail": masktail,
        "i128": np.eye(P, dtype=f16),
    }


def combine_outputs(results):
    # results: list of 8 dicts with "out" [S, D]; partial sums per batch pair
    out = np.empty((4, S, D), np.float32)
    for b in range(4):
        out[b] = results[2 * b]["out"] + results[2 * b + 1]["out"]
    return out


_CACHED_NC = None
last_result = None  # BassKernelResults of the most recent run (for test harness)


def kernel(q, k, v, mask, u, v_bias, Wq, Wk, Wv, Wr, Wo, R):
    global _CACHED_NC, last_result
    from concourse.bass_utils import run_bass_kernel_spmd

    q, k, v = np.asarray(q), np.asarray(k), np.asarray(v)
    u, v_bias = np.asarray(u), np.asarray(v_bias)
    Wq, Wk, Wv, Wr, Wo, R = map(np.asarray, (Wq, Wk, Wv, Wr, Wo, R))

    # The kernel exploits the known TXL mask structure (j <= i + MEM).
    # Verify the passed mask matches; structural masking is baked in.
    m = np.asarray(mask)
    exp_mask = (np.arange(T)[None, :] <= np.arange(S)[:, None] + 1024)
    assert m.shape == (4, S, T) and bool((m == exp_mask[None]).all()), \
        "kernel compiled for the TXL causal mask (j <= i + MEM)"

    if _CACHED_NC is None:
        _CACHED_NC = build_nc()

    in_maps = [prep_core_inputs(c, q, k, v, u, v_bias, Wq, Wk, Wv, Wr, Wo, R)
               for c in range(8)]
    trace = bool(os.environ.get("TXL_TRACE"))
    kwargs = {}
    if trace:
        kwargs = {"trace": True, "tmpdir": os.environ.get("TXL_TRACE_DIR")}
    last_result = run_bass_kernel_spmd(_CACHED_NC, in_maps, list(range(8)), **kwargs)
    return combine_outputs(last_result.results)
